# revision 9
# baseline (speedup 1.0000x reference)
"""DeepSeek-V3 MoE gate (sigmoid scoring + group-limited top-k routing) on 8 trn2 cores.

Token-parallel SPMD: each of the 8 cores routes 2048 of the 16384 tokens with the
full [256, 7168] gate weight + [256] bias replicated.

Per-core pipeline (per 128-token tile):
  1. DMA x tile [128, 7168] to SBUF
  2. PE-transpose x into [d, t] chunks (56 transposes of [128,128], fp32)
  3. 56 accumulating fp32 matmuls -> logits PSUM [128 tok, 256 experts]
  4. ACT sigmoid -> scores, DMA out
  5. DVE routing: +bias, per-group top-2 sums, top-4 group threshold mask,
     top-8 via DVE max/max_index, per-k score gather via compare+reduce,
     renormalize * 2.5
"""

import sys

import numpy as np

if "/opt/trn_rl_repo" not in sys.path:
    sys.path.insert(0, "/opt/trn_rl_repo")

from contextlib import ExitStack

import concourse.bass as bass
import concourse.bacc as bacc
import concourse.tile as tile
from concourse import masks, mybir
from concourse.bass_utils import run_bass_kernel_spmd

F32 = mybir.dt.float32
I32 = mybir.dt.int32
U32 = mybir.dt.uint32
AF = mybir.ActivationFunctionType
ALU = mybir.AluOpType

T_TOTAL = 16384
D = 7168
N = 256
N_CORES = 8
T_CORE = T_TOTAL // N_CORES  # 2048
G = 8           # expert groups
EG = N // G     # experts per group (32)
KC = D // 128   # contraction chunks (56)
TOPK_GROUPS = 4
K_ACT = 8
ROUTE_SCALE = 2.5
NEG_BIG = -1.0e30


def build_nc(t_core: int = T_CORE) -> bass.Bass:
    assert t_core % 128 == 0
    n_tiles = t_core // 128

    nc = bacc.Bacc("TRN2", target_bir_lowering=False, debug=False, enable_asserts=False)
    x_d = nc.dram_tensor("x", [t_core, D], F32, kind="ExternalInput").ap()
    w_d = nc.dram_tensor("weight", [N, D], F32, kind="ExternalInput").ap()
    b_d = nc.dram_tensor("bias", [N], F32, kind="ExternalInput").ap()
    wout_d = nc.dram_tensor("weights_out", [t_core, K_ACT], F32, kind="ExternalOutput").ap()
    iout_d = nc.dram_tensor("indices_out", [t_core, K_ACT], I32, kind="ExternalOutput").ap()
    sout_d = nc.dram_tensor("scores_out", [t_core, N], F32, kind="ExternalOutput").ap()

    QW = 4                 # quarter-splits for x / W loads (fine-grained DMA deps)
    DQ = D // QW           # 1792 columns per quarter
    CQ = KC // QW          # 14 contraction chunks per quarter

    with tile.TileContext(nc) as tc, ExitStack() as ctx:
        const_pool = ctx.enter_context(tc.tile_pool(name="const", bufs=1))
        x_pool = ctx.enter_context(tc.tile_pool(name="x", bufs=2 * QW))
        w_pool = ctx.enter_context(tc.tile_pool(name="w", bufs=2))
        xt_pool = ctx.enter_context(tc.tile_pool(name="xt", bufs=2 * QW))
        sc_pool = ctx.enter_context(tc.tile_pool(name="sc", bufs=2))
        small_pool = ctx.enter_context(tc.tile_pool(name="small", bufs=2))
        psum_mm = ctx.enter_context(tc.tile_pool(name="psmm", bufs=2, space="PSUM"))
        psum_tr = ctx.enter_context(tc.tile_pool(name="pstr", bufs=4, space="PSUM"))

        ident = const_pool.tile([128, 128], F32)
        masks.make_identity(nc, ident[:])
        bias_bc = const_pool.tile([128, N], F32)
        nc.sync.dma_start(bias_bc[:], b_d.unsqueeze(0).broadcast_to((128, N)))
        # per-partition expert-id row [0..255], used for index-equality gather
        iota_i = const_pool.tile([128, N], I32)
        nc.gpsimd.iota(iota_i[:], pattern=[[1, N]], base=0, channel_multiplier=0)
        iota_f = const_pool.tile([128, N], F32)
        nc.vector.tensor_copy(iota_f[:], iota_i[:])

        # Kick off tile-0 x quarters first so the PE has transpose work ASAP.
        def load_x_quarters(i):
            qs = []
            for q in range(QW):
                xq = x_pool.tile([128, DQ], F32, tag="x")
                nc.sync.dma_start(
                    xq[:], x_d[i * 128 : (i + 1) * 128, q * DQ : (q + 1) * DQ]
                )
                qs.append(xq)
            return qs

        x_quarters = load_x_quarters(0)

        # Resident transposed gate weight: wT[:, c*N + e] = weight[e, c*128 + p]
        wT = const_pool.tile([128, KC * N], F32)
        for h in range(2):  # two 128-expert halves
            for q in range(QW):
                w_sb = w_pool.tile([128, DQ], F32, tag="w")
                nc.sync.dma_start(
                    w_sb[:], w_d[h * 128 : (h + 1) * 128, q * DQ : (q + 1) * DQ]
                )
                for lc in range(CQ):
                    c = q * CQ + lc
                    pt = psum_tr.tile([128, 128], F32, tag="pstr")
                    nc.tensor.transpose(pt[:], w_sb[:, lc * 128 : (lc + 1) * 128], ident[:])
                    nc.scalar.activation(
                        wT[:, c * N + h * 128 : c * N + h * 128 + 128], pt[:], AF.Copy
                    )

        for i in range(n_tiles):
            tok = slice(i * 128, (i + 1) * 128)

            xT = []
            for q in range(QW):
                xTq = xt_pool.tile([128, DQ], F32, tag="xt")
                for lc in range(CQ):
                    pt = psum_tr.tile([128, 128], F32, tag="pstr")
                    nc.tensor.transpose(
                        pt[:], x_quarters[q][:, lc * 128 : (lc + 1) * 128], ident[:]
                    )
                    nc.scalar.activation(xTq[:, lc * 128 : (lc + 1) * 128], pt[:], AF.Copy)
                xT.append(xTq)

            # prefetch next tile's x while this tile's matmuls run
            if i + 1 < n_tiles:
                x_quarters = load_x_quarters(i + 1)

            lg = psum_mm.tile([128, N], F32, tag="lg")
            for c in range(KC):
                nc.tensor.matmul(
                    lg[:],
                    xT[c // CQ][:, (c % CQ) * 128 : (c % CQ + 1) * 128],
                    wT[:, c * N : (c + 1) * N],
                    start=(c == 0),
                    stop=(c == KC - 1),
                )

            scores = sc_pool.tile([128, N], F32, tag="scores")
            nc.scalar.activation(scores[:], lg[:], AF.Sigmoid)
            nc.sync.dma_start(sout_d[tok, :], scores[:])

            routing = sc_pool.tile([128, N], F32, tag="routing")
            nc.vector.tensor_add(routing[:], scores[:], bias_bc[:])

            # top-2 per group -> group scores
            g8 = small_pool.tile([128, G * 8], F32, tag="g8")
            for g in range(G):
                nc.vector.max(g8[:, g * 8 : (g + 1) * 8], routing[:, g * EG : (g + 1) * EG])
            g83 = g8[:].rearrange("p (g k) -> p g k", k=8)
            gsum = small_pool.tile([128, G], F32, tag="gsum")
            nc.vector.tensor_add(gsum[:], g83[:, :, 0], g83[:, :, 1])

            # top-4 groups: threshold at 4th largest group score
            gtop = small_pool.tile([128, 8], F32, tag="gtop")
            nc.vector.max(gtop[:], gsum[:])
            keep = small_pool.tile([128, G], F32, tag="keep")
            nc.vector.tensor_scalar(
                keep[:], gsum[:], gtop[:, TOPK_GROUPS - 1 : TOPK_GROUPS], None, op0=ALU.is_ge
            )
            pen = small_pool.tile([128, G], F32, tag="pen")
            nc.vector.tensor_scalar(pen[:], keep[:], -NEG_BIG, NEG_BIG, op0=ALU.mult, op1=ALU.add)
            maskd = sc_pool.tile([128, N], F32, tag="maskd")
            for g in range(G):
                nc.vector.tensor_scalar(
                    maskd[:, g * EG : (g + 1) * EG],
                    routing[:, g * EG : (g + 1) * EG],
                    pen[:, g : g + 1],
                    None,
                    op0=ALU.add,
                )

            # final top-8 with indices
            top8v = small_pool.tile([128, 8], F32, tag="top8v")
            nc.vector.max(top8v[:], maskd[:])
            top8i = small_pool.tile([128, 8], U32, tag="top8i")
            nc.vector.max_index(top8i[:], top8v[:], maskd[:])

            # gather un-biased scores at the top-8 indices (index-equality one-hot:
            # exact even when two routing values collide in fp32)
            idxf = small_pool.tile([128, K_ACT], F32, tag="idxf")
            nc.vector.tensor_copy(idxf[:], top8i[:])
            w8 = small_pool.tile([128, K_ACT], F32, tag="w8")
            eq = sc_pool.tile([128, N], F32, tag="eq")
            scr = sc_pool.tile([128, N], F32, tag="scr")
            for k in range(K_ACT):
                nc.vector.tensor_scalar(
                    eq[:], iota_f[:], idxf[:, k : k + 1], None, op0=ALU.is_equal
                )
                nc.vector.tensor_mul(scr[:], eq[:], scores[:])
                nc.vector.tensor_reduce(
                    w8[:, k : k + 1], scr[:], axis=mybir.AxisListType.X, op=ALU.add
                )

            # renormalize: w = w / clip(sum(w), 1e-9) * ROUTE_SCALE
            denom = small_pool.tile([128, 1], F32, tag="denom")
            nc.vector.tensor_reduce(denom[:], w8[:], axis=mybir.AxisListType.X, op=ALU.add)
            recip = small_pool.tile([128, 1], F32, tag="recip")
            nc.vector.tensor_scalar(recip[:], denom[:], 1e-9, None, op0=ALU.max)
            nc.vector.reciprocal(recip[:], recip[:])
            wout = small_pool.tile([128, K_ACT], F32, tag="wout")
            nc.vector.tensor_scalar(
                wout[:], w8[:], recip[:, 0:1], ROUTE_SCALE, op0=ALU.mult, op1=ALU.mult
            )
            idx32 = small_pool.tile([128, K_ACT], I32, tag="idx32")
            nc.vector.tensor_copy(idx32[:], top8i[:])

            nc.sync.dma_start(wout_d[tok, :], wout[:])
            nc.sync.dma_start(iout_d[tok, :], idx32[:])

    nc.compile()
    return nc


_nc_cache: dict[int, bass.Bass] = {}


def _get_nc(t_core: int) -> bass.Bass:
    if t_core not in _nc_cache:
        _nc_cache[t_core] = build_nc(t_core)
    return _nc_cache[t_core]


def run_spmd(x: np.ndarray, weight: np.ndarray, bias: np.ndarray, **kwargs):
    """Run the SPMD kernel; returns (outputs_tuple, BassKernelResults)."""
    x = np.ascontiguousarray(np.asarray(x), dtype=np.float32)
    weight = np.ascontiguousarray(np.asarray(weight), dtype=np.float32)
    bias = np.ascontiguousarray(np.asarray(bias), dtype=np.float32)
    assert x.shape == (T_TOTAL, D) and weight.shape == (N, D) and bias.shape == (N,)

    nc = _get_nc(T_CORE)
    in_maps = [
        {"x": x[i * T_CORE : (i + 1) * T_CORE], "weight": weight, "bias": bias}
        for i in range(N_CORES)
    ]
    res = run_bass_kernel_spmd(nc, in_maps, list(range(N_CORES)), **kwargs)
    results = res.results
    weights = np.concatenate([results[i]["weights_out"] for i in range(N_CORES)], axis=0)
    indices = np.concatenate([results[i]["indices_out"] for i in range(N_CORES)], axis=0)
    scores = np.concatenate([results[i]["scores_out"] for i in range(N_CORES)], axis=0)
    return (weights, indices.astype(np.int32), scores), res


def kernel(x, weight, bias):
    outs, _ = run_spmd(x, weight, bias)
    return outs


# revision 10
# speedup vs baseline: 1.0848x; 1.0848x over previous
"""DeepSeek-V3 MoE gate (sigmoid scoring + group-limited top-k routing) on 8 trn2 cores.

Token-parallel SPMD: each of the 8 cores routes 2048 of the 16384 tokens with the
full [256, 7168] gate weight + [256] bias replicated.

Per-core pipeline (per 128-token tile):
  1. DMA x tile [128, 7168] to SBUF
  2. PE-transpose x into [d, t] chunks (56 transposes of [128,128], fp32)
  3. 56 accumulating fp32 matmuls -> logits PSUM [128 tok, 256 experts]
  4. ACT sigmoid -> scores, DMA out
  5. DVE routing: +bias, per-group top-2 sums, top-4 group threshold mask,
     top-8 via DVE max/max_index, per-k score gather via compare+reduce,
     renormalize * 2.5
"""

import sys

import numpy as np

if "/opt/trn_rl_repo" not in sys.path:
    sys.path.insert(0, "/opt/trn_rl_repo")

from contextlib import ExitStack

import concourse.bass as bass
import concourse.bacc as bacc
import concourse.tile as tile
from concourse import masks, mybir
from concourse.bass_utils import run_bass_kernel_spmd

F32 = mybir.dt.float32
I32 = mybir.dt.int32
U32 = mybir.dt.uint32
AF = mybir.ActivationFunctionType
ALU = mybir.AluOpType

T_TOTAL = 16384
D = 7168
N = 256
N_CORES = 8
T_CORE = T_TOTAL // N_CORES  # 2048
G = 8           # expert groups
EG = N // G     # experts per group (32)
KC = D // 128   # contraction chunks (56)
TOPK_GROUPS = 4
K_ACT = 8
ROUTE_SCALE = 2.5
NEG_BIG = -1.0e30


def build_nc(t_core: int = T_CORE) -> bass.Bass:
    assert t_core % 128 == 0
    n_tiles = t_core // 128

    nc = bacc.Bacc("TRN2", target_bir_lowering=False, debug=False, enable_asserts=False)
    x_d = nc.dram_tensor("x", [t_core, D], F32, kind="ExternalInput").ap()
    w_d = nc.dram_tensor("weight", [N, D], F32, kind="ExternalInput").ap()
    b_d = nc.dram_tensor("bias", [N], F32, kind="ExternalInput").ap()
    wout_d = nc.dram_tensor("weights_out", [t_core, K_ACT], F32, kind="ExternalOutput").ap()
    iout_d = nc.dram_tensor("indices_out", [t_core, K_ACT], I32, kind="ExternalOutput").ap()
    sout_d = nc.dram_tensor("scores_out", [t_core, N], F32, kind="ExternalOutput").ap()

    QW = 4                 # quarter-splits for x / W loads (fine-grained DMA deps)
    DQ = D // QW           # 1792 columns per quarter
    CQ = KC // QW          # 14 contraction chunks per quarter

    with tile.TileContext(nc) as tc, ExitStack() as ctx:
        const_pool = ctx.enter_context(tc.tile_pool(name="const", bufs=1))
        x_pool = ctx.enter_context(tc.tile_pool(name="x", bufs=2 * QW))
        w_pool = ctx.enter_context(tc.tile_pool(name="w", bufs=2))
        xt_pool = ctx.enter_context(tc.tile_pool(name="xt", bufs=2 * QW))
        sc_pool = ctx.enter_context(tc.tile_pool(name="sc", bufs=2))
        small_pool = ctx.enter_context(tc.tile_pool(name="small", bufs=2))
        psum_mm = ctx.enter_context(tc.tile_pool(name="psmm", bufs=2, space="PSUM"))
        psum_tr = ctx.enter_context(tc.tile_pool(name="pstr", bufs=4, space="PSUM"))

        ident = const_pool.tile([128, 128], F32)
        masks.make_identity(nc, ident[:])
        bias_bc = const_pool.tile([128, N], F32)
        nc.sync.dma_start(bias_bc[:], b_d.unsqueeze(0).broadcast_to((128, N)))
        # per-partition expert-id row [0..255], used for index-equality gather
        iota_i = const_pool.tile([128, N], I32)
        nc.gpsimd.iota(iota_i[:], pattern=[[1, N]], base=0, channel_multiplier=0)
        iota_f = const_pool.tile([128, N], F32)
        nc.vector.tensor_copy(iota_f[:], iota_i[:])

        # Kick off tile-0 x quarters first so the PE has transpose work ASAP.
        def load_x_quarters(i):
            qs = []
            for q in range(QW):
                xq = x_pool.tile([128, DQ], F32, tag="x")
                nc.sync.dma_start(
                    xq[:], x_d[i * 128 : (i + 1) * 128, q * DQ : (q + 1) * DQ]
                )
                qs.append(xq)
            return qs

        def transpose_x_tile(x_quarters):
            xT = []
            for q in range(QW):
                xTq = xt_pool.tile([128, DQ], F32, tag="xt")
                for lc in range(CQ):
                    pt = psum_tr.tile([128, 128], F32, tag="pstr")
                    nc.tensor.transpose(
                        pt[:], x_quarters[q][:, lc * 128 : (lc + 1) * 128], ident[:]
                    )
                    nc.scalar.activation(xTq[:, lc * 128 : (lc + 1) * 128], pt[:], AF.Copy)
                xT.append(xTq)
            return xT

        # tile 0: x load + transposes first, so the PE starts ~3us in;
        # the W DMAs stream in behind them on other queues.
        x_quarters = load_x_quarters(0)
        xT = transpose_x_tile(x_quarters)

        # Resident transposed gate weight: wT[:, c*N + e] = weight[e, c*128 + p]
        wT = const_pool.tile([128, KC * N], F32)
        for h in range(2):  # two 128-expert halves
            for q in range(QW):
                w_sb = w_pool.tile([128, DQ], F32, tag="w")
                nc.sync.dma_start(
                    w_sb[:], w_d[h * 128 : (h + 1) * 128, q * DQ : (q + 1) * DQ]
                )
                for lc in range(CQ):
                    c = q * CQ + lc
                    pt = psum_tr.tile([128, 128], F32, tag="pstr")
                    nc.tensor.transpose(pt[:], w_sb[:, lc * 128 : (lc + 1) * 128], ident[:])
                    nc.scalar.activation(
                        wT[:, c * N + h * 128 : c * N + h * 128 + 128], pt[:], AF.Copy
                    )

        for i in range(n_tiles):
            tok = slice(i * 128, (i + 1) * 128)

            # prefetch next tile's x; its transposes are emitted after this
            # tile's matmuls so the PE stream stays dense
            if i + 1 < n_tiles:
                x_quarters = load_x_quarters(i + 1)

            lg = psum_mm.tile([128, N], F32, tag="lg")
            for c in range(KC):
                nc.tensor.matmul(
                    lg[:],
                    xT[c // CQ][:, (c % CQ) * 128 : (c % CQ + 1) * 128],
                    wT[:, c * N : (c + 1) * N],
                    start=(c == 0),
                    stop=(c == KC - 1),
                )

            if i + 1 < n_tiles:
                xT = transpose_x_tile(x_quarters)

            scores = sc_pool.tile([128, N], F32, tag="scores")
            nc.scalar.activation(scores[:], lg[:], AF.Sigmoid)
            nc.sync.dma_start(sout_d[tok, :], scores[:])

            routing = sc_pool.tile([128, N], F32, tag="routing")
            nc.vector.tensor_add(routing[:], scores[:], bias_bc[:])

            # top-2 per group -> group scores
            g8 = small_pool.tile([128, G * 8], F32, tag="g8")
            for g in range(G):
                nc.vector.max(g8[:, g * 8 : (g + 1) * 8], routing[:, g * EG : (g + 1) * EG])
            g83 = g8[:].rearrange("p (g k) -> p g k", k=8)
            gsum = small_pool.tile([128, G], F32, tag="gsum")
            nc.vector.tensor_add(gsum[:], g83[:, :, 0], g83[:, :, 1])

            # top-4 groups: threshold at 4th largest group score
            gtop = small_pool.tile([128, 8], F32, tag="gtop")
            nc.vector.max(gtop[:], gsum[:])
            keep = small_pool.tile([128, G], F32, tag="keep")
            nc.vector.tensor_scalar(
                keep[:], gsum[:], gtop[:, TOPK_GROUPS - 1 : TOPK_GROUPS], None, op0=ALU.is_ge
            )
            pen = small_pool.tile([128, G], F32, tag="pen")
            nc.vector.tensor_scalar(pen[:], keep[:], -NEG_BIG, NEG_BIG, op0=ALU.mult, op1=ALU.add)
            maskd = sc_pool.tile([128, N], F32, tag="maskd")
            for g in range(G):
                nc.vector.tensor_scalar(
                    maskd[:, g * EG : (g + 1) * EG],
                    routing[:, g * EG : (g + 1) * EG],
                    pen[:, g : g + 1],
                    None,
                    op0=ALU.add,
                )

            # final top-8 with indices
            top8v = small_pool.tile([128, 8], F32, tag="top8v")
            nc.vector.max(top8v[:], maskd[:])
            top8i = small_pool.tile([128, 8], U32, tag="top8i")
            nc.vector.max_index(top8i[:], top8v[:], maskd[:])

            # gather un-biased scores at the top-8 indices (index-equality one-hot:
            # exact even when two routing values collide in fp32)
            idxf = small_pool.tile([128, K_ACT], F32, tag="idxf")
            nc.vector.tensor_copy(idxf[:], top8i[:])
            w8 = small_pool.tile([128, K_ACT], F32, tag="w8")
            eq = sc_pool.tile([128, N], F32, tag="eq")
            scr = sc_pool.tile([128, N], F32, tag="scr")
            for k in range(K_ACT):
                nc.vector.tensor_scalar(
                    eq[:], iota_f[:], idxf[:, k : k + 1], None, op0=ALU.is_equal
                )
                nc.vector.tensor_mul(scr[:], eq[:], scores[:])
                nc.vector.tensor_reduce(
                    w8[:, k : k + 1], scr[:], axis=mybir.AxisListType.X, op=ALU.add
                )

            # renormalize: w = w / clip(sum(w), 1e-9) * ROUTE_SCALE
            denom = small_pool.tile([128, 1], F32, tag="denom")
            nc.vector.tensor_reduce(denom[:], w8[:], axis=mybir.AxisListType.X, op=ALU.add)
            recip = small_pool.tile([128, 1], F32, tag="recip")
            nc.vector.tensor_scalar(recip[:], denom[:], 1e-9, None, op0=ALU.max)
            nc.vector.reciprocal(recip[:], recip[:])
            wout = small_pool.tile([128, K_ACT], F32, tag="wout")
            nc.vector.tensor_scalar(
                wout[:], w8[:], recip[:, 0:1], ROUTE_SCALE, op0=ALU.mult, op1=ALU.mult
            )
            idx32 = small_pool.tile([128, K_ACT], I32, tag="idx32")
            nc.vector.tensor_copy(idx32[:], top8i[:])

            nc.sync.dma_start(wout_d[tok, :], wout[:])
            nc.sync.dma_start(iout_d[tok, :], idx32[:])

    nc.compile()
    return nc


_nc_cache: dict[int, bass.Bass] = {}


def _get_nc(t_core: int) -> bass.Bass:
    if t_core not in _nc_cache:
        _nc_cache[t_core] = build_nc(t_core)
    return _nc_cache[t_core]


def run_spmd(x: np.ndarray, weight: np.ndarray, bias: np.ndarray, **kwargs):
    """Run the SPMD kernel; returns (outputs_tuple, BassKernelResults)."""
    x = np.ascontiguousarray(np.asarray(x), dtype=np.float32)
    weight = np.ascontiguousarray(np.asarray(weight), dtype=np.float32)
    bias = np.ascontiguousarray(np.asarray(bias), dtype=np.float32)
    assert x.shape == (T_TOTAL, D) and weight.shape == (N, D) and bias.shape == (N,)

    nc = _get_nc(T_CORE)
    in_maps = [
        {"x": x[i * T_CORE : (i + 1) * T_CORE], "weight": weight, "bias": bias}
        for i in range(N_CORES)
    ]
    res = run_bass_kernel_spmd(nc, in_maps, list(range(N_CORES)), **kwargs)
    results = res.results
    weights = np.concatenate([results[i]["weights_out"] for i in range(N_CORES)], axis=0)
    indices = np.concatenate([results[i]["indices_out"] for i in range(N_CORES)], axis=0)
    scores = np.concatenate([results[i]["scores_out"] for i in range(N_CORES)], axis=0)
    return (weights, indices.astype(np.int32), scores), res


def kernel(x, weight, bias):
    outs, _ = run_spmd(x, weight, bias)
    return outs


# revision 15
# speedup vs baseline: 1.0862x; 1.0013x over previous
"""DeepSeek-V3 MoE gate (sigmoid scoring + group-limited top-k routing) on 8 trn2 cores.

Token-parallel SPMD: each of the 8 cores routes 2048 of the 16384 tokens with the
full [256, 7168] gate weight + [256] bias replicated.

Per-core pipeline (per 128-token tile):
  1. DMA x tile [128, 7168] to SBUF
  2. PE-transpose x into [d, t] chunks (56 transposes of [128,128], fp32)
  3. 56 accumulating fp32 matmuls -> logits PSUM [128 tok, 256 experts]
  4. ACT sigmoid -> scores, DMA out
  5. DVE routing: +bias, per-group top-2 sums, top-4 group threshold mask,
     top-8 via DVE max/max_index, per-k score gather via compare+reduce,
     renormalize * 2.5
"""

import sys

import numpy as np

if "/opt/trn_rl_repo" not in sys.path:
    sys.path.insert(0, "/opt/trn_rl_repo")

from contextlib import ExitStack

import concourse.bass as bass
import concourse.bacc as bacc
import concourse.tile as tile
from concourse import masks, mybir
from concourse.bass_utils import run_bass_kernel_spmd

F32 = mybir.dt.float32
I32 = mybir.dt.int32
U32 = mybir.dt.uint32
AF = mybir.ActivationFunctionType
ALU = mybir.AluOpType

T_TOTAL = 16384
D = 7168
N = 256
N_CORES = 8
T_CORE = T_TOTAL // N_CORES  # 2048
G = 8           # expert groups
EG = N // G     # experts per group (32)
KC = D // 128   # contraction chunks (56)
TOPK_GROUPS = 4
K_ACT = 8
ROUTE_SCALE = 2.5
NEG_BIG = -1.0e30


def build_nc(t_core: int = T_CORE) -> bass.Bass:
    assert t_core % 128 == 0
    n_tiles = t_core // 128

    nc = bacc.Bacc("TRN2", target_bir_lowering=False, debug=False, enable_asserts=False)
    x_d = nc.dram_tensor("x", [t_core, D], F32, kind="ExternalInput").ap()
    w_d = nc.dram_tensor("weight", [N, D], F32, kind="ExternalInput").ap()
    b_d = nc.dram_tensor("bias", [N], F32, kind="ExternalInput").ap()
    wout_d = nc.dram_tensor("weights_out", [t_core, K_ACT], F32, kind="ExternalOutput").ap()
    iout_d = nc.dram_tensor("indices_out", [t_core, K_ACT], I32, kind="ExternalOutput").ap()
    sout_d = nc.dram_tensor("scores_out", [t_core, N], F32, kind="ExternalOutput").ap()

    QW = 4                 # quarter-splits for x / W loads (fine-grained DMA deps)
    DQ = D // QW           # 1792 columns per quarter
    CQ = KC // QW          # 14 contraction chunks per quarter

    with tile.TileContext(nc) as tc, ExitStack() as ctx:
        const_pool = ctx.enter_context(tc.tile_pool(name="const", bufs=1))
        x_pool = ctx.enter_context(tc.tile_pool(name="x", bufs=2 * QW))
        w_pool = ctx.enter_context(tc.tile_pool(name="w", bufs=2))
        xt_pool = ctx.enter_context(tc.tile_pool(name="xt", bufs=2 * QW))
        sc_pool = ctx.enter_context(tc.tile_pool(name="sc", bufs=2))
        small_pool = ctx.enter_context(tc.tile_pool(name="small", bufs=2))
        psum_mm = ctx.enter_context(tc.tile_pool(name="psmm", bufs=2, space="PSUM"))
        psum_tr = ctx.enter_context(tc.tile_pool(name="pstr", bufs=6, space="PSUM"))

        ident = const_pool.tile([128, 128], F32)
        masks.make_identity(nc, ident[:])
        bias_bc = const_pool.tile([128, N], F32)
        nc.sync.dma_start(bias_bc[:], b_d.unsqueeze(0).broadcast_to((128, N)))
        # per-partition expert-id row [0..255], used for index-equality gather
        iota_i = const_pool.tile([128, N], I32)
        nc.gpsimd.iota(iota_i[:], pattern=[[1, N]], base=0, channel_multiplier=0)
        iota_f = const_pool.tile([128, N], F32)
        nc.vector.tensor_copy(iota_f[:], iota_i[:])

        # Kick off tile-0 x quarters first so the PE has transpose work ASAP.
        # Each quarter may itself be loaded in `splits` sub-DMAs (finer deps for
        # tile 0, where the first transpose gates the whole PE stream).
        def load_x_quarters(i, splits=1):
            qs = []
            for q in range(QW):
                xq = x_pool.tile([128, DQ], F32, tag="x")
                step = DQ // splits
                for s in range(splits):
                    nc.sync.dma_start(
                        xq[:, s * step : (s + 1) * step],
                        x_d[
                            i * 128 : (i + 1) * 128,
                            q * DQ + s * step : q * DQ + (s + 1) * step,
                        ],
                    )
                qs.append(xq)
            return qs

        def transpose_x_tile(x_quarters):
            xT = []
            for q in range(QW):
                xTq = xt_pool.tile([128, DQ], F32, tag="xt")
                for lc in range(CQ):
                    pt = psum_tr.tile([128, 128], F32, tag="pstr")
                    nc.tensor.transpose(
                        pt[:], x_quarters[q][:, lc * 128 : (lc + 1) * 128], ident[:]
                    )
                    nc.scalar.activation(xTq[:, lc * 128 : (lc + 1) * 128], pt[:], AF.Copy)
                xT.append(xTq)
            return xT

        # tile 0: x load + transposes first, so the PE starts ~3us in;
        # the W DMAs stream in behind them on other queues.
        x_quarters = load_x_quarters(0, splits=7)
        xT = transpose_x_tile(x_quarters)

        # Resident transposed gate weight: wT[:, c*N + e] = weight[e, c*128 + p]
        wT = const_pool.tile([128, KC * N], F32)
        for h in range(2):  # two 128-expert halves
            for q in range(QW):
                w_sb = w_pool.tile([128, DQ], F32, tag="w")
                nc.sync.dma_start(
                    w_sb[:], w_d[h * 128 : (h + 1) * 128, q * DQ : (q + 1) * DQ]
                )
                for lc in range(CQ):
                    c = q * CQ + lc
                    pt = psum_tr.tile([128, 128], F32, tag="pstr")
                    nc.tensor.transpose(pt[:], w_sb[:, lc * 128 : (lc + 1) * 128], ident[:])
                    nc.scalar.activation(
                        wT[:, c * N + h * 128 : c * N + h * 128 + 128], pt[:], AF.Copy
                    )

        for i in range(n_tiles):
            tok = slice(i * 128, (i + 1) * 128)

            # prefetch next tile's x; its transposes are emitted after this
            # tile's matmuls so the PE stream stays dense
            if i + 1 < n_tiles:
                x_quarters = load_x_quarters(i + 1)

            lg = psum_mm.tile([128, N], F32, tag="lg")
            for c in range(KC):
                nc.tensor.matmul(
                    lg[:],
                    xT[c // CQ][:, (c % CQ) * 128 : (c % CQ + 1) * 128],
                    wT[:, c * N : (c + 1) * N],
                    start=(c == 0),
                    stop=(c == KC - 1),
                )

            if i + 1 < n_tiles:
                xT = transpose_x_tile(x_quarters)

            scores = sc_pool.tile([128, N], F32, tag="scores")
            nc.scalar.activation(scores[:], lg[:], AF.Sigmoid)
            nc.sync.dma_start(sout_d[tok, :], scores[:])

            routing = sc_pool.tile([128, N], F32, tag="routing")
            nc.vector.tensor_add(routing[:], scores[:], bias_bc[:])

            # top-2 per group -> group scores
            g8 = small_pool.tile([128, G * 8], F32, tag="g8")
            for g in range(G):
                nc.vector.max(g8[:, g * 8 : (g + 1) * 8], routing[:, g * EG : (g + 1) * EG])
            g83 = g8[:].rearrange("p (g k) -> p g k", k=8)
            gsum = small_pool.tile([128, G], F32, tag="gsum")
            nc.vector.tensor_add(gsum[:], g83[:, :, 0], g83[:, :, 1])

            # top-4 groups: threshold at 4th largest group score
            gtop = small_pool.tile([128, 8], F32, tag="gtop")
            nc.vector.max(gtop[:], gsum[:])
            keep = small_pool.tile([128, G], F32, tag="keep")
            nc.vector.tensor_scalar(
                keep[:], gsum[:], gtop[:, TOPK_GROUPS - 1 : TOPK_GROUPS], None, op0=ALU.is_ge
            )
            pen = small_pool.tile([128, G], F32, tag="pen")
            nc.vector.tensor_scalar(pen[:], keep[:], -NEG_BIG, NEG_BIG, op0=ALU.mult, op1=ALU.add)
            maskd = sc_pool.tile([128, N], F32, tag="maskd")
            for g in range(G):
                nc.vector.tensor_scalar(
                    maskd[:, g * EG : (g + 1) * EG],
                    routing[:, g * EG : (g + 1) * EG],
                    pen[:, g : g + 1],
                    None,
                    op0=ALU.add,
                )

            # final top-8 with indices
            top8v = small_pool.tile([128, 8], F32, tag="top8v")
            nc.vector.max(top8v[:], maskd[:])
            top8i = small_pool.tile([128, 8], U32, tag="top8i")
            nc.vector.max_index(top8i[:], top8v[:], maskd[:])

            # gather un-biased scores at the top-8 indices (index-equality one-hot:
            # exact even when two routing values collide in fp32)
            idxf = small_pool.tile([128, K_ACT], F32, tag="idxf")
            nc.vector.tensor_copy(idxf[:], top8i[:])
            w8 = small_pool.tile([128, K_ACT], F32, tag="w8")
            eq = sc_pool.tile([128, N], F32, tag="eq")
            scr = sc_pool.tile([128, N], F32, tag="scr")
            for k in range(K_ACT):
                nc.vector.tensor_scalar(
                    eq[:], iota_f[:], idxf[:, k : k + 1], None, op0=ALU.is_equal
                )
                nc.vector.tensor_mul(scr[:], eq[:], scores[:])
                nc.vector.tensor_reduce(
                    w8[:, k : k + 1], scr[:], axis=mybir.AxisListType.X, op=ALU.add
                )

            # renormalize: w = w / clip(sum(w), 1e-9) * ROUTE_SCALE
            denom = small_pool.tile([128, 1], F32, tag="denom")
            nc.vector.tensor_reduce(denom[:], w8[:], axis=mybir.AxisListType.X, op=ALU.add)
            recip = small_pool.tile([128, 1], F32, tag="recip")
            nc.vector.tensor_scalar(recip[:], denom[:], 1e-9, None, op0=ALU.max)
            nc.vector.reciprocal(recip[:], recip[:])
            wout = small_pool.tile([128, K_ACT], F32, tag="wout")
            nc.vector.tensor_scalar(
                wout[:], w8[:], recip[:, 0:1], ROUTE_SCALE, op0=ALU.mult, op1=ALU.mult
            )
            idx32 = small_pool.tile([128, K_ACT], I32, tag="idx32")
            nc.vector.tensor_copy(idx32[:], top8i[:])

            nc.sync.dma_start(wout_d[tok, :], wout[:])
            nc.sync.dma_start(iout_d[tok, :], idx32[:])

    nc.compile()
    return nc


_nc_cache: dict[int, bass.Bass] = {}


def _get_nc(t_core: int) -> bass.Bass:
    if t_core not in _nc_cache:
        _nc_cache[t_core] = build_nc(t_core)
    return _nc_cache[t_core]


def run_spmd(x: np.ndarray, weight: np.ndarray, bias: np.ndarray, **kwargs):
    """Run the SPMD kernel; returns (outputs_tuple, BassKernelResults)."""
    x = np.ascontiguousarray(np.asarray(x), dtype=np.float32)
    weight = np.ascontiguousarray(np.asarray(weight), dtype=np.float32)
    bias = np.ascontiguousarray(np.asarray(bias), dtype=np.float32)
    assert x.shape == (T_TOTAL, D) and weight.shape == (N, D) and bias.shape == (N,)

    nc = _get_nc(T_CORE)
    in_maps = [
        {"x": x[i * T_CORE : (i + 1) * T_CORE], "weight": weight, "bias": bias}
        for i in range(N_CORES)
    ]
    res = None
    last_err = None
    for attempt in range(3):
        try:
            res = run_bass_kernel_spmd(nc, in_maps, list(range(N_CORES)), **kwargs)
            break
        except Exception as e:  # transient NRT_EXEC_UNIT_UNRECOVERABLE after prior crashes
            last_err = e
            import time as _time

            _time.sleep(5)
    if res is None:
        raise last_err
    results = res.results
    weights = np.concatenate([results[i]["weights_out"] for i in range(N_CORES)], axis=0)
    indices = np.concatenate([results[i]["indices_out"] for i in range(N_CORES)], axis=0)
    scores = np.concatenate([results[i]["scores_out"] for i in range(N_CORES)], axis=0)
    return (weights, indices.astype(np.int32), scores), res


def kernel(x, weight, bias):
    outs, _ = run_spmd(x, weight, bias)
    return outs
